# revision 19
# baseline (speedup 1.0000x reference)
import sys
from contextlib import ExitStack

import numpy as np

sys.path.insert(0, "/opt/trn_rl_repo")

# Problem constants (hardcoded per contract)
N_NODES = 50000
N_EDGES = 1600000
G = 32        # EDGE_FEAT
HID = 64      # EDGE_HIDDEN
H = 128       # NODE_FEAT
CORES = 8
K = 64        # edge slots per node (max in-degree for this input distribution)
NPC = 6272    # padded nodes per core (49 * 128)
NT = NPC // 128


def _build_bass(debug_taps=False):
    from concourse import bacc, mybir
    import concourse.tile as tile

    dt = mybir.dt.float32
    AF = mybir.ActivationFunctionType
    AX = mybir.AxisListType
    OP = mybir.AluOpType

    nc_obj = bacc.Bacc(
        "TRN2", target_bir_lowering=False, debug=False,
        enable_asserts=False, num_devices=CORES,
    )

    CW = K * G + K + H  # combo row: [pf | plog | nf]
    combo_d = nc_obj.dram_tensor("combo", [NPC, CW], dt, kind="ExternalInput").ap()
    weT_d = nc_obj.dram_tensor("weT", [G, HID], dt, kind="ExternalInput").ap()
    be_d = nc_obj.dram_tensor("be", [HID, 1], dt, kind="ExternalInput").ap()
    nbe_d = nc_obj.dram_tensor("nbe", [HID, 1], dt, kind="ExternalInput").ap()
    # wihT_aug: rows 0..63 = W_ih.T ; row 64 = bias row (rz: b_ih+b_hh-colsum,
    # n: b_ih_n+0.5*b_hh_n-colsum) so K=65 matmul adds all gi-side biases.
    wihT_d = nc_obj.dram_tensor("wihT", [HID + 1, 3 * H], dt, kind="ExternalInput").ap()
    # whhT_mod: cols 0:256 = W_hh.T (rz) ; cols 256:384 = 0.5 * W_hh.T (n)
    whhT_d = nc_obj.dram_tensor("whhT", [H, 3 * H], dt, kind="ExternalInput").ap()
    bhhn_d = nc_obj.dram_tensor("bhhn", [1, H], dt, kind="ExternalInput").ap()
    ones_d = nc_obj.dram_tensor("ones1", [1, H], dt, kind="ExternalInput").ap()
    ident_d = nc_obj.dram_tensor("ident", [H, H], dt, kind="ExternalInput").ap()
    hout_d = nc_obj.dram_tensor("hout", [NPC, H], dt, kind="ExternalOutput").ap()
    if debug_taps:
        y_dbg = nc_obj.dram_tensor("y_dbg", [NPC, G], dt, kind="ExternalOutput").ap()

    with tile.TileContext(nc_obj) as tc, ExitStack() as ctx:
        nc = tc.nc
        cpool = ctx.enter_context(tc.tile_pool(name="consts", bufs=1))
        weT = cpool.tile([G, HID], dt, tag="weT")
        nc.sync.dma_start(weT[:], weT_d)
        be = cpool.tile([HID, 1], dt, tag="be")
        nc.sync.dma_start(be[:], be_d)
        nbe = cpool.tile([HID, 1], dt, tag="nbe")
        nc.sync.dma_start(nbe[:], nbe_d)
        wihT = cpool.tile([HID + 1, 3 * H], dt, tag="wihT")
        nc.sync.dma_start(wihT[:], wihT_d)
        whhT = cpool.tile([H, 3 * H], dt, tag="whhT")
        nc.sync.dma_start(whhT[:], whhT_d)
        bhhn = cpool.tile([1, H], dt, tag="bhhn")
        nc.sync.dma_start(bhhn[:], bhhn_d)
        ones1 = cpool.tile([1, H], dt, tag="ones1")
        nc.sync.dma_start(ones1[:], ones_d)
        ident = cpool.tile([H, H], dt, tag="ident")
        nc.sync.dma_start(ident[:], ident_d)

        inp = ctx.enter_context(tc.tile_pool(name="inp", bufs=4))
        mid = ctx.enter_context(tc.tile_pool(name="mid", bufs=3))
        pp = ctx.enter_context(tc.tile_pool(name="pp", bufs=1, space="PSUM"))
        outp = ctx.enter_context(tc.tile_pool(name="outp", bufs=3))

        for i in range(NT):
            r0 = i * 128
            combo = inp.tile([128, CW], dt, tag="combo")
            nc.sync.dma_start(combo[:], combo_d[r0:r0 + 128, :])
            pf = combo[:, 0:K * G]
            pl = combo[:, K * G:K * G + K]
            nft_t = inp.tile([128, H], dt, tag="nfc")
            nc.gpsimd.tensor_copy(nft_t[:], combo[:, K * G + K:])
            nft = nft_t[:]

            # ex = exp(logits); S = row-sum(ex) fused into the activation
            ex = mid.tile([128, K], dt, tag="ex")
            S = mid.tile([128, 1], dt, tag="S")
            nc.scalar.activation(ex[:], pl, AF.Exp, accum_out=S[:])
            Sc = mid.tile([128, 1], dt, tag="Sc")
            nc.gpsimd.tensor_scalar_max(Sc[:], S[:], 1e-30)
            rS = mid.tile([128, 1], dt, tag="rS")
            nc.vector.reciprocal(rS[:], Sc[:])

            # w[n, j, g] = pf[n, j, g] * ex[n, j]
            w = mid.tile([128, K * G], dt, tag="w")
            exb = ex[:].rearrange("p (j o) -> p j o", o=1).broadcast_to([128, K, G])
            nc.vector.tensor_tensor(
                w[:].rearrange("p (j g) -> p j g", g=G),
                pf.rearrange("p (j g) -> p j g", g=G),
                exb, op=OP.mult,
            )
            # y[n, g] = sum_j w[n, j, g]
            y = mid.tile([128, G], dt, tag="y")
            nc.vector.reduce_sum(
                y[:], w[:].rearrange("p (j g) -> p g j", g=G), axis=AX.X
            )
            yn = mid.tile([128, G], dt, tag="yn")
            nc.vector.tensor_scalar_mul(yn[:], y[:], rS[:])

            # transpose yn -> [G, 128]
            ynT_ps = pp.tile([G, 128], dt, tag="ynT")
            nc.tensor.transpose(ynT_ps[:], yn[:], ident[:])
            ynT = mid.tile([G, 128], dt, tag="ynTs")
            nc.scalar.copy(ynT[:], ynT_ps[:])

            # cT = W_e @ ynT  -> [HID, 128]
            cT_ps = pp.tile([HID, 128], dt, tag="cT")
            nc.tensor.matmul(cT_ps[:], weT[:], ynT[:], start=True, stop=True)

            # ctx' = elu(c+be) + 1 = relu(c+be) + exp(min(c+be, 0)); the +1
            # offset is compensated in wihT's bias row (colsum subtracted).
            rn = mid.tile([HID, 128], dt, tag="rn")
            nc.scalar.activation(rn[:], cT_ps[:], AF.Relu, bias=be[:])
            mn = mid.tile([HID, 128], dt, tag="mn")
            nc.scalar.activation(mn[:], cT_ps[:], AF.Relu, bias=nbe[:], scale=-1.0)
            en = mid.tile([HID, 128], dt, tag="en")
            nc.scalar.activation(en[:], mn[:], AF.Exp, scale=-1.0)
            ctxT = mid.tile([HID + 1, 128], dt, tag="ctxT")
            nc.gpsimd.tensor_add(ctxT[0:HID, :], rn[:], en[:])
            nc.gpsimd.memset(ctxT[HID:HID + 1, :], 1.0)

            nfT_ps = pp.tile([H, 128], dt, tag="nfT")
            nc.tensor.transpose(nfT_ps[:], nft, ident[:])
            nfT = mid.tile([H, 128], dt, tag="nfTs")
            nc.scalar.copy(nfT[:], nfT_ps[:])

            # gates: tr/tz = tanh(0.5*(gi+gh)) ; sigmoid(x) = (1+tanh(x/2))/2
            rz_ps = pp.tile([128, 2 * H], dt, tag="rz")
            nc.tensor.matmul(rz_ps[:], ctxT[:], wihT[:, 0:2 * H], start=True, stop=False)
            nc.tensor.matmul(rz_ps[:], nfT[:], whhT[:, 0:2 * H], start=False, stop=True)
            # ni = gi_n + 0.5*gh_n (bias in aug row); nh = 0.5*gh_n + 0.5*b_hh_n
            ni_ps = pp.tile([128, H], dt, tag="ni")
            nc.tensor.matmul(ni_ps[:], ctxT[:], wihT[:, 2 * H:], start=True, stop=False)
            nc.tensor.matmul(ni_ps[:], nfT[:], whhT[:, 2 * H:], start=False, stop=True)
            nh_ps = pp.tile([128, H], dt, tag="nh")
            nc.tensor.matmul(nh_ps[:], nfT[:], whhT[:, 2 * H:], start=True, stop=False)
            nc.tensor.matmul(nh_ps[:], ones1[:], bhhn[:], start=False, stop=True)

            trz = mid.tile([128, 2 * H], dt, tag="trz")
            nc.scalar.activation(trz[:], rz_ps[:], AF.Tanh, scale=0.5)
            tr = trz[:, 0:H]
            tz = trz[:, H:2 * H]

            t1 = mid.tile([128, H], dt, tag="t1")
            nc.vector.tensor_mul(t1[:], tr[:], nh_ps[:])
            t2 = mid.tile([128, H], dt, tag="t2")
            nc.vector.tensor_add(t2[:], t1[:], ni_ps[:])
            n_t = mid.tile([128, H], dt, tag="n")
            nc.scalar.activation(n_t[:], t2[:], AF.Tanh)
            # h = 0.5*(n + nf + tz*(nf - n)); relu(h) = relu-with-scale
            d_t = mid.tile([128, H], dt, tag="d")
            nc.gpsimd.tensor_sub(d_t[:], nft, n_t[:])
            zd = mid.tile([128, H], dt, tag="zd")
            nc.gpsimd.tensor_mul(zd[:], tz[:], d_t[:])
            s1 = mid.tile([128, H], dt, tag="s1")
            nc.gpsimd.tensor_add(s1[:], n_t[:], nft)
            hp = mid.tile([128, H], dt, tag="hp")
            nc.gpsimd.tensor_add(hp[:], s1[:], zd[:])
            ho = outp.tile([128, H], dt, tag="ho")
            nc.scalar.activation(ho[:], hp[:], AF.Relu, scale=0.5)
            nc.sync.dma_start(hout_d[r0:r0 + 128, :], ho[:])
            if debug_taps:
                nc.sync.dma_start(y_dbg[r0:r0 + 128, :], y[:])

    nc_obj.compile()
    return nc_obj


_NC_CACHE = None


def kernel(**inputs):
    global _NC_CACHE
    from concourse.bass_utils import run_bass_kernel_spmd

    el = np.ascontiguousarray(np.asarray(inputs["edge_logits"], np.float32)[:, 0])
    ef = np.ascontiguousarray(np.asarray(inputs["edge_feats"], np.float32))
    nf = np.asarray(inputs["node_feats"], np.float32)
    dst = np.asarray(inputs["dst"]).astype(np.int64)
    W_e = np.asarray(inputs["W_e"], np.float32)
    b_e = np.asarray(inputs["b_e"], np.float32)
    W_ih = np.asarray(inputs["W_ih"], np.float32)
    W_hh = np.asarray(inputs["W_hh"], np.float32)
    b_ih = np.asarray(inputs["b_ih"], np.float32)
    b_hh = np.asarray(inputs["b_hh"], np.float32)

    # ---- host-side layout: stable-sort edges by dst, pad per node to K slots
    order = np.argsort(dst, kind="stable")
    sd = dst[order]
    counts = np.bincount(sd, minlength=N_NODES)
    assert counts.max() <= K, f"max in-degree {counts.max()} > {K}"
    starts = np.zeros(N_NODES + 1, np.int64)
    np.cumsum(counts, out=starts[1:])
    NPAD = NPC * CORES
    rank = np.arange(N_EDGES, dtype=np.int64) - starts[sd]
    pos = sd * K + rank
    plog = np.full((NPAD * K,), -1e30, np.float32)
    plog[pos] = el[order]
    pfeat = np.zeros((NPAD * K, G), np.float32)
    pfeat[pos] = ef[order]
    nfp = np.zeros((NPAD, H), np.float32)
    nfp[:N_NODES] = nf

    combo = np.concatenate(
        [
            pfeat.reshape(NPAD, K * G),
            plog.reshape(NPAD, K),
            nfp,
        ],
        axis=1,
    ).reshape(CORES, NPC, K * G + K + H)

    # weight prep (pure affine rearrangement of the reference weights)
    wihT = np.ascontiguousarray(W_ih.T)                    # [64, 384]
    colsum = wihT.sum(axis=0)                              # [384]
    bias_row = np.empty((3 * H,), np.float32)
    bias_row[:2 * H] = b_ih[:2 * H] + b_hh[:2 * H] - colsum[:2 * H]
    bias_row[2 * H:] = b_ih[2 * H:] + 0.5 * b_hh[2 * H:] - colsum[2 * H:]
    wihT_aug = np.vstack([wihT, bias_row[None, :]])        # [65, 384]
    whhT_mod = np.ascontiguousarray(W_hh.T).copy()         # [128, 384]
    whhT_mod[:, 2 * H:] *= 0.5
    bhhn = (0.5 * b_hh[2 * H:]).reshape(1, H)

    common = {
        "weT": np.ascontiguousarray(W_e.T),
        "be": b_e.reshape(HID, 1).copy(),
        "nbe": (-b_e).reshape(HID, 1).copy(),
        "wihT": wihT_aug,
        "whhT": whhT_mod,
        "bhhn": bhhn.astype(np.float32),
        "ones1": np.ones((1, H), np.float32),
        "ident": np.eye(H, dtype=np.float32),
    }
    in_maps = [dict(combo=combo[c], **common) for c in range(CORES)]

    if _NC_CACHE is None:
        _NC_CACHE = _build_bass()
    res = run_bass_kernel_spmd(_NC_CACHE, in_maps, core_ids=list(range(CORES)))
    out = np.concatenate(
        [res.results[c]["hout"] for c in range(CORES)], axis=0
    )[:N_NODES]
    return out.astype(np.float32)
